# revision 1
# baseline (speedup 1.0000x reference)
"""GAT (graph attention) message-passing kernel for Trainium2, 8 NeuronCores.

Strategy: edges sharded by destination node across cores. Host relabels nodes
(degree-balanced dealing into 128-node dst blocks) so every block has ~equal
edge count. Device phase 1 computes per-node projections h and src-scores into
a gather table (replicated per core). Phase 2 processes dst blocks: gathers
h-rows of edge sources (chunked int16 indexed dma_gather), computes softmax
weights, and aggregates via onehot-matmul into PSUM, then projects with W_out.
"""
import sys

sys.path.insert(0, "/opt/trn_rl_repo")

import numpy as np

from concourse import bacc, bass, mybir, tile
from concourse.bass_utils import run_bass_kernel_spmd

f32 = mybir.dt.float32
i16 = mybir.dt.int16
i32 = mybir.dt.int32
AF = mybir.ActivationFunctionType
ALU = mybir.AluOpType

N = 100000
E = 1600000
D = 128            # in dim
H = 4              # heads
HD = 32            # head dim
OUTD = 128
NEG = 0.2
EPS = 1e-8

NCORES = 8
BLK_PER_CORE = 98
NB_G = NCORES * BLK_PER_CORE      # 784 global blocks
NPAD = NB_G * 128                 # 100352 padded nodes
NPB = BLK_PER_CORE * 128          # 12544 nodes per core
NCHUNK = 4
CH = NPAD // NCHUNK               # 25088 rows per gather chunk (< 32768)
ROW = 192                         # table row: [h(128) | s_src(4) | pad] f32 = 768B


# ---------------------------------------------------------------- host prep
def _host_prep(x, edge_index, mask, W, a_src, a_dst, W_out):
    src = np.asarray(edge_index[0], np.int64)
    dst = np.asarray(edge_index[1], np.int64)
    m = np.asarray(mask, bool)
    keep = m[src]
    src, dst = src[keep], dst[keep]

    # node relabeling: deal nodes (sorted by in-degree desc) snake-wise into
    # NB_G blocks so block edge counts are balanced
    deg = np.bincount(dst, minlength=N)
    order = np.argsort(-deg, kind="stable")
    r = np.arange(N)
    rounds = r // NB_G
    pos = r % NB_G
    blk_of_rank = np.where(rounds % 2 == 0, pos, NB_G - 1 - pos)
    pi = np.empty(N, np.int64)
    pi[order] = blk_of_rank * 128 + rounds

    nsrc = pi[src]
    ndst = pi[dst]
    core = ndst // NPB
    b_loc = (ndst % NPB) // 128
    seg = ndst % 128
    ch = nsrc // CH
    loc = nsrc % CH

    # per (core, block, chunk) edge counts
    gid = (core * BLK_PER_CORE + b_loc) * NCHUNK + ch
    counts = np.bincount(gid, minlength=NB_G * NCHUNK).reshape(
        NCORES, BLK_PER_CORE, NCHUNK
    )
    caps = counts.max(axis=0)  # [BLK_PER_CORE, NCHUNK]
    caps = np.maximum(((caps + 127) // 128) * 128, 128).astype(np.int64)

    blk_slots = caps.sum(axis=1)              # [BLK_PER_CORE]
    blk_off = np.concatenate([[0], np.cumsum(blk_slots)])
    tot = int(blk_off[-1])
    grp_off = np.zeros((BLK_PER_CORE, NCHUNK), np.int64)
    for b in range(BLK_PER_CORE):
        o = blk_off[b]
        for c in range(NCHUNK):
            grp_off[b, c] = o
            o += caps[b, c]

    # slot assignment per core
    idx_flat = np.zeros((NCORES, tot), np.int16)      # pad -> row 0
    seg_flat = np.full((NCORES, tot), 128.0, np.float32)  # pad -> seg 128
    ordr = np.lexsort((loc, ch, b_loc, core))
    core_s, b_s, ch_s, loc_s, seg_s = (
        core[ordr], b_loc[ordr], ch[ordr], loc[ordr], seg[ordr]
    )
    # position of each edge within its (core, block, chunk) group
    gkey = (core_s * BLK_PER_CORE + b_s) * NCHUNK + ch_s
    # edges are sorted by gkey; rank within group:
    first = np.concatenate([[True], gkey[1:] != gkey[:-1]])
    gstart = np.flatnonzero(first)
    grp_len = np.diff(np.concatenate([gstart, [len(gkey)]]))
    rank = np.arange(len(gkey)) - np.repeat(gstart, grp_len)
    slot = grp_off[b_s, ch_s] + rank
    idx_flat[core_s, slot] = loc_s.astype(np.int16)
    seg_flat[core_s, slot] = seg_s.astype(np.float32)

    # device layouts
    # idxs: per (block, chunk) wrap cap idxs -> [16, cap/16] -> tile to [128, cap/16]
    idx_dev = np.zeros((NCORES, 128, tot // 16), np.int16)
    # segs: slot i -> [i%128, i//128] within block
    seg_dev = np.zeros((NCORES, 128, tot // 128), np.float32)
    # segfm: seg value per slot, replicated across 128 partitions (bf16)
    import jax.numpy as jnp
    segfm_dev = np.asarray(jnp.asarray(seg_flat, jnp.bfloat16))  # [NCORES, tot]
    segfm_rep = [np.ascontiguousarray(np.broadcast_to(segfm_dev[c][None, :], (128, tot)))
                 for c in range(NCORES)]
    for b in range(BLK_PER_CORE):
        for c in range(NCHUNK):
            o = grp_off[b, c]
            cap = caps[b, c]
            chunk_idx = idx_flat[:, o : o + cap]                 # [8, cap]
            wrap = chunk_idx.reshape(NCORES, cap // 16, 16).transpose(0, 2, 1)
            idx_dev[:, :, o // 16 : (o + cap) // 16] = np.tile(wrap, (1, 8, 1))
        o = blk_off[b]
        sl = seg_flat[:, o : o + blk_slots[b]]
        seg_dev[:, :, o // 128 : (o + blk_slots[b]) // 128] = sl.reshape(
            NCORES, blk_slots[b] // 128, 128
        ).transpose(0, 2, 1)

    # xT padded and permuted: column pi[n] holds x[n]  (bf16 for device matmuls)
    xT32 = np.zeros((D, NPAD), np.float32)
    xT32[:, pi] = np.asarray(x, np.float32).T
    import jax.numpy as jnp
    xT = np.asarray(jnp.asarray(xT32, jnp.bfloat16))

    # weights
    W_cat = np.asarray(W, np.float32).transpose(1, 0, 2).reshape(D, H * HD)
    A_src = np.zeros((H * HD, H), np.float32)
    A_dst = np.zeros((H * HD, H), np.float32)
    for h in range(H):
        A_src[h * HD : (h + 1) * HD, h] = np.asarray(a_src, np.float32)[h]
        A_dst[h * HD : (h + 1) * HD, h] = np.asarray(a_dst, np.float32)[h]
    M_src = W_cat @ A_src            # [128, 4]
    M_dst = W_cat @ A_dst            # [128, 4]
    wcat_ext = np.concatenate([W_cat, M_src], axis=1)  # [128, 132]

    meta = dict(
        caps=caps, blk_off=blk_off, grp_off=grp_off, blk_slots=blk_slots,
        tot=tot, pi=pi,
    )
    wcat_b = np.asarray(jnp.asarray(wcat_ext, jnp.bfloat16))
    mdst_b = np.asarray(jnp.asarray(M_dst, jnp.bfloat16))
    wout_b = np.asarray(jnp.asarray(np.asarray(W_out, np.float32), jnp.bfloat16))
    ident_b = np.asarray(jnp.asarray(np.eye(128, dtype=np.float32), jnp.bfloat16))
    per_core = []
    for c in range(NCORES):
        per_core.append(
            dict(
                xT=xT,
                xTd=np.ascontiguousarray(xT[:, c * NPB : (c + 1) * NPB]),
                wcat_ext=wcat_b,
                mdst=mdst_b,
                wout=wout_b,
                ident=ident_b,
                idxs=idx_dev[c],
                segs=seg_dev[c],
                segfm=segfm_rep[c],
            )
        )
    return per_core, meta


# ---------------------------------------------------------------- device build
def _build_nc(meta):
    caps = meta["caps"]
    blk_off = meta["blk_off"]
    grp_off = meta["grp_off"]
    blk_slots = meta["blk_slots"]
    tot = meta["tot"]
    bf16 = mybir.dt.bfloat16

    nc = bacc.Bacc(None, target_bir_lowering=False)
    xT = nc.dram_tensor("xT", [D, NPAD], bf16, kind="ExternalInput")
    xTd = nc.dram_tensor("xTd", [D, NPB], bf16, kind="ExternalInput")
    wcat_ext = nc.dram_tensor("wcat_ext", [D, 132], bf16, kind="ExternalInput")
    mdst = nc.dram_tensor("mdst", [D, H], bf16, kind="ExternalInput")
    wout = nc.dram_tensor("wout", [H * HD, OUTD], bf16, kind="ExternalInput")
    ident = nc.dram_tensor("ident", [128, 128], bf16, kind="ExternalInput")
    idxs = nc.dram_tensor("idxs", [128, tot // 16], i16, kind="ExternalInput")
    segs = nc.dram_tensor("segs", [128, tot // 128], f32, kind="ExternalInput")
    segfm = nc.dram_tensor("segfm", [128, tot], bf16, kind="ExternalInput")
    out = nc.dram_tensor("out", [NPB, OUTD], f32, kind="ExternalOutput")
    table = nc.dram_tensor("table", [NPAD, 256], bf16, kind="Internal")

    n_t1 = NPAD // 128  # phase-1 tiles

    with tile.TileContext(nc) as tc:
        with (
            tc.tile_pool(name="const", bufs=1) as cpool,
            tc.tile_pool(name="p1", bufs=4) as p1,
            tc.tile_pool(name="gath", bufs=2) as gp,
            tc.tile_pool(name="work", bufs=3) as wp,
            tc.tile_pool(name="outp", bufs=3) as op_,
            tc.tile_pool(name="ps1", bufs=2, space="PSUM") as ps1,
            tc.tile_pool(name="psB", bufs=2, space="PSUM") as psB,
            tc.tile_pool(name="psS", bufs=1, space="PSUM") as psS,
            tc.tile_pool(name="psT", bufs=1, space="PSUM") as psT,
            tc.tile_pool(name="psE", bufs=1, space="PSUM") as psE,
        ):
            # constants
            wcat_sb = cpool.tile([D, 132], bf16)
            nc.sync.dma_start(wcat_sb[:, :], wcat_ext[:, :])
            mdst_sb = cpool.tile([D, H], bf16)
            nc.sync.dma_start(mdst_sb[:, :], mdst[:, :])
            wout_sb = cpool.tile([H * HD, OUTD], bf16)
            nc.sync.dma_start(wout_sb[:, :], wout[:, :])
            ident_sb = cpool.tile([128, 128], bf16)
            nc.sync.dma_start(ident_sb[:, :], ident[:, :])
            iota_i = cpool.tile([128, 128], i32)
            nc.gpsimd.iota(iota_i[:, :], pattern=[[1, 128]], base=0,
                           channel_multiplier=0)
            iota_b = cpool.tile([128, 128], bf16)
            nc.vector.tensor_copy(iota_b[:, :], iota_i[:, :])
            iotac_i = cpool.tile([128, 1], i32)
            nc.gpsimd.iota(iotac_i[:, :], pattern=[[0, 1]], base=0,
                           channel_multiplier=1)
            iotac_f = cpool.tile([128, 1], f32)
            nc.vector.tensor_copy(iotac_f[:, :], iotac_i[:, :])

            # ---------------- phase 1: table[n] = [h(128) | s_src(4)] ----------
            for i in range(n_t1):
                xt_t = p1.tile([128, 128], bf16, tag="xt")
                nc.sync.dma_start(xt_t[:, :], xT[:, i * 128 : (i + 1) * 128])
                ps = ps1.tile([128, 132], f32, tag="ps1")
                nc.tensor.matmul(ps[:, :], xt_t[:, :], wcat_sb[:, :],
                                 start=True, stop=True)
                row = p1.tile([128, 132], bf16, tag="row")
                nc.vector.tensor_copy(row[:, :], ps[:, :])
                nc.sync.dma_start(table[i * 128 : (i + 1) * 128, 0:132], row[:, :])

            # ---------------- phase 2: per dst block --------------------------
            for b in range(BLK_PER_CORE):
                nt = int(blk_slots[b]) // 128  # tiles in this block
                o16 = int(blk_off[b]) // 16
                o128 = int(blk_off[b]) // 128

                it = wp.tile([128, blk_slots[b] // 16], i16, tag="it")
                nc.sync.dma_start(it[:, :], idxs[:, o16 : o16 + blk_slots[b] // 16])
                sg = wp.tile([128, nt], f32, tag="sg")
                nc.sync.dma_start(sg[:, :], segs[:, o128 : o128 + nt])
                sfm = wp.tile([128, blk_slots[b]], bf16, tag="sfm")
                nc.sync.dma_start(
                    sfm[:, :],
                    segfm[:, blk_off[b] : blk_off[b] + blk_slots[b]],
                )

                # s_dst for this block of 128 dst nodes
                xtd_t = wp.tile([128, 128], bf16, tag="xtd")
                nc.sync.dma_start(xtd_t[:, :], xTd[:, b * 128 : (b + 1) * 128])
                ps_sd = psS.tile([128, H], f32, tag="sd")
                nc.tensor.matmul(ps_sd[:, :], xtd_t[:, :], mdst_sb[:, :],
                                 start=True, stop=True)
                sdst_b = wp.tile([128, H], bf16, tag="sdst")
                nc.vector.tensor_copy(sdst_b[:, :], ps_sd[:, :])

                # gather table rows for all slots (4 chunks)
                G = gp.tile([128, nt, 256], bf16, tag="G")
                for c in range(NCHUNK):
                    cap = int(caps[b, c])
                    go = (int(grp_off[b, c]) - int(blk_off[b])) // 128
                    nc.gpsimd.dma_gather(
                        out_ap=G[:, go : go + cap // 128, :],
                        in_ap=table[c * CH : (c + 1) * CH, :],
                        idxs_ap=it[:, (int(grp_off[b, c]) - int(blk_off[b])) // 16 :
                                   (int(grp_off[b, c]) - int(blk_off[b]) + cap) // 16],
                        num_idxs=cap,
                        num_idxs_reg=cap,
                        elem_size=256,
                    )

                # ohT[j, slot] = (seg(slot) == j), one batched op per block
                ohT = wp.tile([128, blk_slots[b]], bf16, tag="ohT")
                nc.vector.tensor_scalar(ohT[:, :], sfm[:, :], iotac_f[:, 0:1],
                                        None, op0=ALU.is_equal)
                # s_dst expanded to slots via PE
                ps_se = psE.tile([128, nt * H], f32, tag="se")
                for t in range(nt):
                    nc.tensor.matmul(ps_se[:, t * H : (t + 1) * H],
                                     ohT[:, t * 128 : (t + 1) * 128],
                                     sdst_b[:, :], start=True, stop=True)

                # scores: w = exp(max(e, NEG*e)), e = s_src + s_dst  [128, nt, H]
                ssrc = wp.tile([128, nt, H], f32, tag="ssrc")
                nc.vector.tensor_copy(ssrc[:, :, :], G[:, :, 128 : 128 + H])
                esum = wp.tile([128, nt, H], f32, tag="esum")
                nc.vector.tensor_tensor(
                    esum[:, :, :], ssrc[:, :, :],
                    ps_se[:, :].rearrange("p (t h) -> p t h", h=H),
                    op=ALU.add,
                )
                e2 = wp.tile([128, nt, H], f32, tag="e2")
                nc.vector.tensor_scalar_mul(e2[:, :, :], esum[:, :, :], NEG)
                lr = wp.tile([128, nt, H], f32, tag="lr")
                nc.vector.tensor_tensor(lr[:, :, :], esum[:, :, :], e2[:, :, :],
                                        op=ALU.max)
                w = wp.tile([128, nt, H], f32, tag="w")
                nc.scalar.activation(w[:, :, :], lr[:, :, :], AF.Exp)
                wb = wp.tile([128, nt, H], bf16, tag="wb")
                nc.vector.tensor_copy(wb[:, :, :], w[:, :, :])

                # G2 = [G * w_bcast | w]  -> [128, nt, 132] bf16
                G2 = gp.tile([128, nt, 132], bf16, tag="G2")
                w_b = wb[:, :, :].unsqueeze(3).broadcast_to((128, nt, H, HD))
                nc.vector.tensor_tensor(
                    G2[:, :, 0:128].rearrange("p t (h k) -> p t h k", h=H),
                    G[:, :, 0:128].rearrange("p t (h k) -> p t h k", h=H),
                    w_b,
                    op=ALU.mult,
                )
                nc.vector.tensor_copy(G2[:, :, 128:132], wb[:, :, :])

                # aggregation: psum[seg, 0:128] = sum alpha*h ; [:,128:132] = Z
                pb = psB.tile([128, 132], f32, tag="pb")
                for t in range(nt):
                    oh = wp.tile([128, 128], bf16, tag="oh")
                    nc.vector.tensor_scalar(
                        oh[:, :], iota_b[:, :], sg[:, t : t + 1], None,
                        op0=ALU.is_equal,
                    )
                    nc.tensor.matmul(pb[:, :], oh[:, :], G2[:, t, :],
                                     start=(t == 0), stop=(t == nt - 1))

                # normalize: na = agg / (Z + eps)
                radd = wp.tile([128, H], f32, tag="radd")
                nc.vector.tensor_scalar_add(radd[:, :], pb[:, 128:132], EPS)
                rec = wp.tile([128, H], f32, tag="rec")
                nc.vector.reciprocal(rec[:, :], radd[:, :])
                na = op_.tile([128, 128], bf16, tag="na")
                nc.vector.tensor_tensor(
                    na[:, :].rearrange("p (h k) -> p h k", h=H),
                    pb[:, 0:128].rearrange("p (h k) -> p h k", h=H),
                    rec[:, :].unsqueeze(2).broadcast_to((128, H, HD)),
                    op=ALU.mult,
                )

                # out rows = (na @ wout): transpose na, then matmul
                pt = psT.tile([128, 128], bf16, tag="pt")
                nc.tensor.transpose(pt[:, :], na[:, :], ident_sb[:, :])
                naT = op_.tile([128, 128], bf16, tag="naT")
                nc.vector.tensor_copy(naT[:, :], pt[:, :])
                po = psT.tile([128, 128], f32, tag="po")
                nc.tensor.matmul(po[:, :], naT[:, :], wout_sb[:, :],
                                 start=True, stop=True)
                ot = op_.tile([128, 128], f32, tag="ot")
                nc.vector.tensor_copy(ot[:, :], po[:, :])
                nc.sync.dma_start(out[b * 128 : (b + 1) * 128, :], ot[:, :])

    nc.compile()
    return nc


# ---------------------------------------------------------------- entry point
def kernel(x, edge_index, mask, W, a_src, a_dst, W_out, _cache={}):
    per_core, meta = _host_prep(x, edge_index, mask, W, a_src, a_dst, W_out)
    key = (meta["tot"], tuple(meta["blk_slots"].tolist()))
    if key not in _cache:
        _cache[key] = _build_nc(meta)
    nc = _cache[key]
    res = run_bass_kernel_spmd(nc, per_core, core_ids=list(range(NCORES)))
    out_new = np.concatenate([res.results[c]["out"] for c in range(NCORES)], axis=0)
    return out_new[meta["pi"]].astype(np.float32)


if __name__ == "__main__":
    rng = np.random.default_rng(0)
    x = rng.standard_normal((N, D)).astype(np.float32)
    ei = rng.integers(0, N, size=(2, E)).astype(np.int32)
    mask = np.ones((N,), bool)
    W = (rng.standard_normal((H, D, HD)) * 0.05).astype(np.float32)
    a_s = (rng.standard_normal((H, HD)) * 0.1).astype(np.float32)
    a_d = (rng.standard_normal((H, HD)) * 0.1).astype(np.float32)
    W_o = (rng.standard_normal((H * HD, OUTD)) * 0.05).astype(np.float32)
    out = kernel(x, ei, mask, W, a_s, a_d, W_o)
    print("ok", out.shape, out.dtype)



# revision 2
# speedup vs baseline: 3.5663x; 3.5663x over previous
"""GAT (graph attention) message-passing kernel for Trainium2, 8 NeuronCores.

v2 strategy: dst-major edge grid, no device-side gather.

Host relabels nodes by descending in-degree and deals them round-robin to the
8 cores, so every core sees the same degree profile. Each core's 12544 dst
nodes form 98 blocks of 128; block b holds nodes of similar degree, padded to
a common per-block edge count K_b (the max degree in the block, identical
across cores). The host pre-gathers x[src] for every edge into a feature-major
DRAM grid xeT[128, TOT] whose column (block, k, seg) is the k-th in-edge of
dst node seg of that block.

Device work per block: one matmul per k-tile projects 128 edges' source
features into [dst, h(128)|s_src(4)] PSUM rows (dst on partitions). Scores,
leaky-relu, exp, masking, and the weighted aggregation are then pure
elementwise/reduction work along the free dim - no one-hot matmuls, no
dma_gather, no GpSimd at all.
"""
import sys

sys.path.insert(0, "/opt/trn_rl_repo")

import numpy as np
import ml_dtypes

from concourse import bacc, mybir, tile
from concourse.bass_utils import run_bass_kernel_spmd

f32 = mybir.dt.float32
bf16d = mybir.dt.bfloat16
i32 = mybir.dt.int32
AF = mybir.ActivationFunctionType
ALU = mybir.AluOpType
BF = ml_dtypes.bfloat16

N = 100000
E = 1600000
D = 128            # in dim
H = 4              # heads
HD = 32            # head dim
OUTD = 128
NEG = 0.2
EPS = 1e-8

NCORES = 8
BLK = 98
NPB = BLK * 128          # 12544 dst nodes per core
NPAD = NCORES * NPB      # 100352
GRP = 3                  # h-matmuls per PSUM bank tile (3*132 words <= 512)
KC = 12                  # k-tiles per score-chunk round


# ---------------------------------------------------------------- host prep
def _host_prep(x, edge_index, mask, W, a_src, a_dst, W_out):
    x = np.asarray(x, np.float32)
    src = np.asarray(edge_index[0], np.int64)
    dst = np.asarray(edge_index[1], np.int64)
    keep = np.asarray(mask, bool)[src]
    if not keep.all():
        src, dst = src[keep], dst[keep]

    deg = np.bincount(dst, minlength=N).astype(np.int64)
    degp = np.full(NPAD, -1, np.int64)
    degp[:N] = deg
    order = np.argsort(-degp, kind="stable")
    rank = np.empty(NPAD, np.int64)
    rank[order] = np.arange(NPAD)
    core_of = rank % NCORES
    pos = rank // NCORES

    deg_sorted = degp[order]
    Kb = np.maximum(deg_sorted[np.arange(BLK) * 128 * NCORES], 1).astype(np.int64)
    assert Kb.max() <= 64, Kb.max()
    colstart = np.concatenate([[0], np.cumsum(Kb * 128)]).astype(np.int64)
    TOT = int(colstart[-1])

    # k-rank of each edge within its dst node
    o2 = np.argsort(dst, kind="stable")
    ds, ss = dst[o2], src[o2]
    first = np.r_[True, ds[1:] != ds[:-1]]
    gstart = np.flatnonzero(first)
    glen = np.diff(np.r_[gstart, [len(ds)]])
    krank = np.arange(len(ds)) - np.repeat(gstart, glen)
    b_e = pos[ds] // 128
    s_e = pos[ds] % 128
    c_e = core_of[ds]
    col = colstart[b_e] + krank * 128 + s_e

    xbf = x.astype(BF)
    node_at = np.full((NCORES, NPB), N, np.int64)   # default: pad slot
    node_at[core_of, pos] = np.arange(NPAD)
    xpad = np.zeros((NPAD, D), np.float32)
    xpad[:N] = x
    degf_pad = np.zeros(NPAD, np.float32)
    degf_pad[:N] = deg

    W_cat = np.asarray(W, np.float32).transpose(1, 0, 2).reshape(D, H * HD)
    A_src = np.zeros((H * HD, H), np.float32)
    A_dst = np.zeros((H * HD, H), np.float32)
    for h in range(H):
        A_src[h * HD:(h + 1) * HD, h] = np.asarray(a_src, np.float32)[h]
        A_dst[h * HD:(h + 1) * HD, h] = np.asarray(a_dst, np.float32)[h]
    wcat_ext = np.ascontiguousarray(
        np.concatenate([W_cat, W_cat @ A_src], axis=1)).astype(BF)  # [128, 132]
    mdst_h = np.ascontiguousarray(W_cat @ A_dst).astype(BF)          # [128, 4]
    wout_h = np.asarray(W_out, np.float32).astype(BF)
    ident_h = np.eye(128, dtype=np.float32).astype(BF)

    per_core = []
    for c in range(NCORES):
        sel = c_e == c
        xeT = np.zeros((D, TOT), BF)
        xeT[:, col[sel]] = xbf[ss[sel]].T
        nodes = node_at[c]
        xtd = np.ascontiguousarray(xpad[nodes].T).astype(BF)           # [128, NPB]
        degf = np.ascontiguousarray(
            degf_pad[nodes].reshape(BLK, 128).T).astype(np.float32)    # [128, BLK]
        per_core.append(dict(
            xeT=xeT, xtd=xtd, degf=degf,
            wcat_ext=wcat_ext, mdst=mdst_h, wout=wout_h, ident=ident_h,
        ))

    pi = (core_of * NPB + pos)[:N]
    meta = dict(Kb=Kb, colstart=colstart, tot=TOT, pi=pi)
    return per_core, meta


# ---------------------------------------------------------------- device build
def _build_nc(meta):
    Kb = [int(k) for k in meta["Kb"]]
    colstart = [int(v) for v in meta["colstart"]]
    TOT = int(meta["tot"])

    nc = bacc.Bacc(None, target_bir_lowering=False)
    xeT = nc.dram_tensor("xeT", [D, TOT], bf16d, kind="ExternalInput")
    xtd = nc.dram_tensor("xtd", [D, NPB], bf16d, kind="ExternalInput")
    degf = nc.dram_tensor("degf", [128, BLK], f32, kind="ExternalInput")
    wcat_ext = nc.dram_tensor("wcat_ext", [D, 132], bf16d, kind="ExternalInput")
    mdst = nc.dram_tensor("mdst", [D, H], bf16d, kind="ExternalInput")
    wout = nc.dram_tensor("wout", [H * HD, OUTD], bf16d, kind="ExternalInput")
    ident = nc.dram_tensor("ident", [128, 128], bf16d, kind="ExternalInput")
    out = nc.dram_tensor("out", [NPB, OUTD], f32, kind="ExternalOutput")

    with tile.TileContext(nc) as tc:
        with (
            tc.tile_pool(name="const", bufs=1) as cpool,
            tc.tile_pool(name="xe", bufs=3) as xp,
            tc.tile_pool(name="work", bufs=2) as wp,
            tc.tile_pool(name="outp", bufs=3) as op_,
            tc.tile_pool(name="psH", bufs=5, space="PSUM") as psH,
            tc.tile_pool(name="psK", bufs=1, space="PSUM") as psK,
            tc.tile_pool(name="psT", bufs=1, space="PSUM") as psT,
        ):
            wcat_sb = cpool.tile([D, 132], bf16d)
            nc.sync.dma_start(wcat_sb[:, :], wcat_ext[:, :])
            mdst_sb = cpool.tile([D, H], bf16d)
            nc.sync.dma_start(mdst_sb[:, :], mdst[:, :])
            wout_sb = cpool.tile([H * HD, OUTD], bf16d)
            nc.sync.dma_start(wout_sb[:, :], wout[:, :])
            ident_sb = cpool.tile([128, 128], bf16d)
            nc.sync.dma_start(ident_sb[:, :], ident[:, :])
            xtd_sb = cpool.tile([D, NPB], bf16d)
            nc.sync.dma_start(xtd_sb[:, :], xtd[:, :])
            degf_sb = cpool.tile([128, BLK], f32)
            nc.sync.dma_start(degf_sb[:, :], degf[:, :])
            iota_i = cpool.tile([128, 64], i32)
            nc.gpsimd.iota(iota_i[:, :], pattern=[[1, 64]], base=0,
                           channel_multiplier=0)
            iota_f = cpool.tile([128, 64], f32)
            nc.vector.tensor_copy(iota_f[:, :], iota_i[:, :])

            for b in range(BLK):
                K = Kb[b]
                c0b = colstart[b]
                xe = xp.tile([128, K * 128], bf16d, tag="xe")
                nc.sync.dma_start(xe[:, :], xeT[:, c0b : c0b + K * 128])

                # s_dst for the block's 128 dst nodes
                ps_sd = psK.tile([128, H], f32, tag="sd")
                nc.tensor.matmul(ps_sd[:, :], xtd_sb[:, b * 128 : (b + 1) * 128],
                                 mdst_sb[:, :], start=True, stop=True)
                sdst = wp.tile([128, H], f32, tag="sdst")
                nc.vector.tensor_copy(sdst[:, :], ps_sd[:, :])

                # mask[seg, k] = k < deg(seg)
                msk = wp.tile([128, K], f32, tag="msk")
                nc.vector.tensor_scalar(msk[:, :], iota_f[:, 0:K],
                                        degf_sb[:, b : b + 1], None, op0=ALU.is_lt)

                ssrc = wp.tile([128, K, H], f32, tag="ssrc")
                esum = wp.tile([128, K, H], f32, tag="esum")
                lr = wp.tile([128, K, H], f32, tag="lr")
                wex = wp.tile([128, K, H], f32, tag="wex")
                wm = wp.tile([128, K, H], f32, tag="wm")
                G2 = wp.tile([128, K, 132], bf16d, tag="G2")

                for c0 in range(0, K, KC):
                    kc = min(KC, K - c0)
                    groups = []
                    for g0 in range(c0, c0 + kc, GRP):
                        m = min(GRP, c0 + kc - g0)
                        ps = psH.tile([128, GRP * 132], f32, tag="ps")
                        for j in range(m):
                            k = g0 + j
                            nc.tensor.matmul(
                                ps[:, j * 132 : (j + 1) * 132],
                                xe[:, k * 128 : (k + 1) * 128],
                                wcat_sb[:, :], start=True, stop=True)
                        psv = ps[:, :].rearrange("p (j f) -> p j f", f=132)
                        nc.vector.tensor_copy(ssrc[:, g0 : g0 + m, :],
                                              psv[:, 0:m, 128:132])
                        groups.append((ps, g0, m))

                    sl = slice(c0, c0 + kc)
                    nc.vector.tensor_tensor(
                        esum[:, sl, :], ssrc[:, sl, :],
                        sdst[:, :].unsqueeze(1).broadcast_to((128, kc, H)),
                        op=ALU.add)
                    nc.vector.scalar_tensor_tensor(
                        lr[:, sl, :], esum[:, sl, :], NEG, esum[:, sl, :],
                        op0=ALU.mult, op1=ALU.max)
                    nc.scalar.activation(wex[:, sl, :], lr[:, sl, :], AF.Exp)
                    nc.vector.tensor_tensor(
                        wm[:, sl, :], wex[:, sl, :],
                        msk[:, sl].unsqueeze(2).broadcast_to((128, kc, H)),
                        op=ALU.mult)

                    for ps, g0, m in groups:
                        psv = ps[:, :].rearrange("p (j f) -> p j f", f=132)
                        w_bc = wm[:, g0 : g0 + m, :].unsqueeze(3).broadcast_to(
                            (128, m, H, HD))
                        nc.vector.tensor_tensor(
                            G2[:, g0 : g0 + m, 0:128].rearrange(
                                "p k (h f) -> p k h f", h=H),
                            psv[:, 0:m, 0:128].rearrange(
                                "p j (h f) -> p j h f", h=H),
                            w_bc, op=ALU.mult)

                # Z, 1/(Z+eps)
                zt = wp.tile([128, H], f32, tag="zt")
                nc.vector.tensor_reduce(zt[:, :],
                                        wm[:, :, :].rearrange("p k h -> p h k"),
                                        mybir.AxisListType.X, ALU.add)
                zr = wp.tile([128, H], f32, tag="zr")
                nc.vector.tensor_scalar_add(zr[:, :], zt[:, :], EPS)
                zrec = wp.tile([128, H], f32, tag="zrec")
                nc.vector.reciprocal(zrec[:, :], zr[:, :])

                # agg[dst, f] = sum_k G2[dst, k, f]
                agg = wp.tile([128, 128], f32, tag="agg")
                nc.vector.tensor_reduce(
                    agg[:, :], G2[:, :, 0:128].rearrange("p k f -> p f k"),
                    mybir.AxisListType.X, ALU.add)

                na = wp.tile([128, 128], bf16d, tag="na")
                nc.vector.tensor_tensor(
                    na[:, :].rearrange("p (h f) -> p h f", h=H),
                    agg[:, :].rearrange("p (h f) -> p h f", h=H),
                    zrec[:, :].unsqueeze(2).broadcast_to((128, H, HD)),
                    op=ALU.mult)

                # out rows = (na @ wout)
                pt = psT.tile([128, 128], bf16d, tag="pt")
                nc.tensor.transpose(pt[:, :], na[:, :], ident_sb[:, :])
                naT = wp.tile([128, 128], bf16d, tag="naT")
                nc.vector.tensor_copy(naT[:, :], pt[:, :])
                po = psT.tile([128, OUTD], f32, tag="po")
                nc.tensor.matmul(po[:, :], naT[:, :], wout_sb[:, :],
                                 start=True, stop=True)
                ot = op_.tile([128, OUTD], f32, tag="ot")
                nc.vector.tensor_copy(ot[:, :], po[:, :])
                nc.sync.dma_start(out[b * 128 : (b + 1) * 128, :], ot[:, :])

    nc.compile()
    return nc


# ---------------------------------------------------------------- entry point
def kernel(x, edge_index, mask, W, a_src, a_dst, W_out, _cache={}):
    per_core, meta = _host_prep(x, edge_index, mask, W, a_src, a_dst, W_out)
    key = (meta["tot"], tuple(int(k) for k in meta["Kb"]))
    if key not in _cache:
        _cache[key] = _build_nc(meta)
    nc = _cache[key]
    res = run_bass_kernel_spmd(nc, per_core, core_ids=list(range(NCORES)))
    out_new = np.concatenate([res.results[c]["out"] for c in range(NCORES)], axis=0)
    return out_new[meta["pi"]].astype(np.float32)


if __name__ == "__main__":
    rng = np.random.default_rng(0)
    x = rng.standard_normal((N, D)).astype(np.float32)
    ei = rng.integers(0, N, size=(2, E)).astype(np.int32)
    mask = np.ones((N,), bool)
    W = (rng.standard_normal((H, D, HD)) * 0.05).astype(np.float32)
    a_s = (rng.standard_normal((H, HD)) * 0.1).astype(np.float32)
    a_d = (rng.standard_normal((H, HD)) * 0.1).astype(np.float32)
    W_o = (rng.standard_normal((H * HD, OUTD)) * 0.05).astype(np.float32)
    out = kernel(x, ei, mask, W, a_s, a_d, W_o)
    print("ok", out.shape, out.dtype)


# revision 5
# speedup vs baseline: 5.1342x; 1.4397x over previous
"""GAT (graph attention) message-passing kernel for Trainium2, 8 NeuronCores.

v5 strategy: dst-major edge grid, no device-side gather, bf16 bulk vector ops.

Host relabels nodes by descending in-degree and deals them round-robin to the
8 cores, so every core sees the same degree profile. Each core's 12544 dst
nodes form 98 blocks of 128; block b holds nodes of similar degree, padded to
a common per-block edge count K_b (the max degree in the block, identical
across cores). The host pre-gathers x[src] for every edge into a feature-major
DRAM grid xeT[128, TOT] whose column (block, k, seg) is the k-th in-edge of
dst node seg of that block. Pad columns are zero, so padded edges contribute
exactly zero to the aggregation; only the softmax normalizer Z needs a
closed-form correction  Z -= (K - deg) * exp(leaky(s_dst)).

Device work per block: one matmul per k-tile projects 128 edges' source
features into [dst, h(128)|s_src(4)] PSUM rows (dst on partitions). The
Scalar engine copies h to SBUF bf16 (freeing PSUM early) and computes exp;
Vector does scores, one whole-block bf16 weighted-message multiply, and a
contiguous halving-tree k-reduction. No one-hot matmuls, no dma_gather,
no GpSimd.
"""
import sys

sys.path.insert(0, "/opt/trn_rl_repo")

import numpy as np
import ml_dtypes

from concourse import bacc, mybir, tile
from concourse.bass_utils import run_bass_kernel_spmd

f32 = mybir.dt.float32
bf16d = mybir.dt.bfloat16
AF = mybir.ActivationFunctionType
ALU = mybir.AluOpType
BF = ml_dtypes.bfloat16

N = 100000
E = 1600000
D = 128            # in dim
H = 4              # heads
HD = 32            # head dim
OUTD = 128
NEG = 0.2
EPS = 1e-8

NCORES = 8
BLK = 98
NPB = BLK * 128          # 12544 dst nodes per core
NPAD = NCORES * NPB      # 100352
GRP = 3                  # h-matmuls per PSUM bank tile (3*132 words <= 512)


# ---------------------------------------------------------------- host prep
def _host_prep(x, edge_index, mask, W, a_src, a_dst, W_out):
    x = np.asarray(x, np.float32)
    src = np.asarray(edge_index[0], np.int64)
    dst = np.asarray(edge_index[1], np.int64)
    keep = np.asarray(mask, bool)[src]
    if not keep.all():
        src, dst = src[keep], dst[keep]

    deg = np.bincount(dst, minlength=N).astype(np.int64)
    degp = np.full(NPAD, -1, np.int64)
    degp[:N] = deg
    order = np.argsort(-degp, kind="stable")
    rank = np.empty(NPAD, np.int64)
    rank[order] = np.arange(NPAD)
    core_of = rank % NCORES
    pos = rank // NCORES

    deg_sorted = degp[order]
    Kb = np.maximum(deg_sorted[np.arange(BLK) * 128 * NCORES], 1).astype(np.int64)
    assert Kb.max() <= 64, Kb.max()
    colstart = np.concatenate([[0], np.cumsum(Kb * 128)]).astype(np.int64)
    TOT = int(colstart[-1])

    # k-rank of each edge within its dst node
    o2 = np.argsort(dst, kind="stable")
    ds, ss = dst[o2], src[o2]
    first = np.r_[True, ds[1:] != ds[:-1]]
    gstart = np.flatnonzero(first)
    glen = np.diff(np.r_[gstart, [len(ds)]])
    krank = np.arange(len(ds)) - np.repeat(gstart, glen)
    b_e = pos[ds] // 128
    s_e = pos[ds] % 128
    c_e = core_of[ds]
    col = colstart[b_e] + krank * 128 + s_e

    xbf = x.astype(BF)
    node_at = np.full((NCORES, NPB), N, np.int64)   # default: pad slot
    node_at[core_of, pos] = np.arange(NPAD)
    xpad = np.zeros((NPAD, D), np.float32)
    xpad[:N] = x
    deg_pad = np.zeros(NPAD, np.float32)
    deg_pad[:N] = deg

    W_cat = np.asarray(W, np.float32).transpose(1, 0, 2).reshape(D, H * HD)
    A_src = np.zeros((H * HD, H), np.float32)
    A_dst = np.zeros((H * HD, H), np.float32)
    for h in range(H):
        A_src[h * HD:(h + 1) * HD, h] = np.asarray(a_src, np.float32)[h]
        A_dst[h * HD:(h + 1) * HD, h] = np.asarray(a_dst, np.float32)[h]
    wcat_ext = np.ascontiguousarray(
        np.concatenate([W_cat, W_cat @ A_src], axis=1)).astype(BF)  # [128, 132]
    mdst_h = np.ascontiguousarray(W_cat @ A_dst).astype(BF)          # [128, 4]
    wout_h = np.asarray(W_out, np.float32).astype(BF)
    ident_h = np.eye(128, dtype=np.float32).astype(BF)

    per_core = []
    for c in range(NCORES):
        sel = c_e == c
        xeT = np.zeros((D, TOT), BF)
        xeT[:, col[sel]] = xbf[ss[sel]].T
        nodes = node_at[c]
        xtd = np.ascontiguousarray(xpad[nodes].T).astype(BF)           # [128, NPB]
        # K_b - deg per (seg, block): pad count for the Z correction
        kmd = (Kb[None, :].astype(np.float32)
               - deg_pad[nodes].reshape(BLK, 128).T)                   # [128, BLK]
        per_core.append(dict(
            xeT=xeT, xtd=xtd, kmd=np.ascontiguousarray(kmd).astype(np.float32),
            wcat_ext=wcat_ext, mdst=mdst_h, wout=wout_h, ident=ident_h,
        ))

    pi = (core_of * NPB + pos)[:N]
    meta = dict(Kb=Kb, colstart=colstart, tot=TOT, pi=pi)
    return per_core, meta


# ---------------------------------------------------------------- device build
def _build_nc(meta):
    Kb = [int(k) for k in meta["Kb"]]
    colstart = [int(v) for v in meta["colstart"]]
    TOT = int(meta["tot"])

    nc = bacc.Bacc(None, target_bir_lowering=False)
    xeT = nc.dram_tensor("xeT", [D, TOT], bf16d, kind="ExternalInput")
    xtd = nc.dram_tensor("xtd", [D, NPB], bf16d, kind="ExternalInput")
    kmd = nc.dram_tensor("kmd", [128, BLK], f32, kind="ExternalInput")
    wcat_ext = nc.dram_tensor("wcat_ext", [D, 132], bf16d, kind="ExternalInput")
    mdst = nc.dram_tensor("mdst", [D, H], bf16d, kind="ExternalInput")
    wout = nc.dram_tensor("wout", [H * HD, OUTD], bf16d, kind="ExternalInput")
    ident = nc.dram_tensor("ident", [128, 128], bf16d, kind="ExternalInput")
    out = nc.dram_tensor("out", [NPB, OUTD], f32, kind="ExternalOutput")

    with tile.TileContext(nc) as tc:
        with (
            tc.tile_pool(name="const", bufs=1) as cpool,
            tc.tile_pool(name="xe", bufs=3) as xp,
            tc.tile_pool(name="work", bufs=2) as wp,
            tc.tile_pool(name="outp", bufs=3) as op_,
            tc.tile_pool(name="psH", bufs=5, space="PSUM") as psH,
            tc.tile_pool(name="psK", bufs=1, space="PSUM") as psK,
            tc.tile_pool(name="psT", bufs=1, space="PSUM") as psT,
        ):
            wcat_sb = cpool.tile([D, 132], bf16d)
            nc.sync.dma_start(wcat_sb[:, :], wcat_ext[:, :])
            mdst_sb = cpool.tile([D, H], bf16d)
            nc.sync.dma_start(mdst_sb[:, :], mdst[:, :])
            wout_sb = cpool.tile([H * HD, OUTD], bf16d)
            nc.sync.dma_start(wout_sb[:, :], wout[:, :])
            ident_sb = cpool.tile([128, 128], bf16d)
            nc.sync.dma_start(ident_sb[:, :], ident[:, :])
            xtd_sb = cpool.tile([D, NPB], bf16d)
            nc.sync.dma_start(xtd_sb[:, :], xtd[:, :])
            kmd_sb = cpool.tile([128, BLK], f32)
            nc.sync.dma_start(kmd_sb[:, :], kmd[:, :])

            for b in range(BLK):
                K = Kb[b]
                c0b = colstart[b]

                xe = xp.tile([128, K * 128], bf16d, tag="xe")
                nc.sync.dma_start(xe[:, :], xeT[:, c0b : c0b + K * 128])

                # s_dst for the block's 128 dst nodes
                ps_sd = psK.tile([128, H], f32, tag="sd")
                nc.tensor.matmul(ps_sd[:, :], xtd_sb[:, b * 128 : (b + 1) * 128],
                                 mdst_sb[:, :], start=True, stop=True)
                sdst = wp.tile([128, H], f32, tag="sdst")
                nc.scalar.activation(sdst[:, :], ps_sd[:, :], AF.Copy)

                esum = wp.tile([128, H, K], f32, tag="esum")
                hcp = wp.tile([128, K, 128], bf16d, tag="hcp")

                for g0 in range(0, K, GRP):
                    m = min(GRP, K - g0)
                    ps = psH.tile([128, GRP * 132], f32, tag="ps")
                    for j in range(m):
                        k = g0 + j
                        nc.tensor.matmul(
                            ps[:, j * 132 : (j + 1) * 132],
                            xe[:, k * 128 : (k + 1) * 128],
                            wcat_sb[:, :], start=True, stop=True)
                    psv = ps[:, :].rearrange("p (j f) -> p j f", f=132)
                    # e = s_src + s_dst, straight off PSUM
                    nc.vector.tensor_tensor(
                        esum[:, :, g0 : g0 + m],
                        psv[:, 0:m, 128:132].rearrange("p j h -> p h j"),
                        sdst[:, :].unsqueeze(2).broadcast_to((128, H, m)),
                        op=ALU.add)
                    # h rows to SBUF bf16 (frees PSUM)
                    nc.scalar.activation(
                        hcp[:, g0 : g0 + m, :], psv[:, 0:m, 0:128], AF.Copy)

                # w = exp(leaky(e)) ; wb = bf16(w) ; Z = sum_k w
                lrt = wp.tile([128, H, K], f32, tag="lrt")
                nc.vector.scalar_tensor_tensor(
                    lrt[:, :, :], esum[:, :, :], NEG, esum[:, :, :],
                    op0=ALU.mult, op1=ALU.max)
                wex = wp.tile([128, H, K], f32, tag="wex")
                nc.scalar.activation(wex[:, :, :], lrt[:, :, :], AF.Exp)
                wexb = wp.tile([128, H, K], bf16d, tag="wexb")
                nc.vector.tensor_copy(wexb[:, :, :], wex[:, :, :])
                zall = wp.tile([128, H], f32, tag="zall")
                nc.vector.tensor_reduce(zall[:, :], wex[:, :, :],
                                        mybir.AxisListType.X, ALU.add)

                # Z -= (K-deg)*exp(leaky(s_dst)); zrec = 1/(Z+eps)
                lrp = wp.tile([128, H], f32, tag="lrp")
                nc.vector.scalar_tensor_tensor(
                    lrp[:, :], sdst[:, :], NEG, sdst[:, :],
                    op0=ALU.mult, op1=ALU.max)
                wpad = wp.tile([128, H], f32, tag="wpad")
                nc.scalar.activation(wpad[:, :], lrp[:, :], AF.Exp)
                corr = wp.tile([128, H], f32, tag="corr")
                nc.vector.tensor_scalar(corr[:, :], wpad[:, :],
                                        kmd_sb[:, b : b + 1], None, op0=ALU.mult)
                zfix = wp.tile([128, H], f32, tag="zfix")
                nc.vector.scalar_tensor_tensor(
                    zfix[:, :], zall[:, :], EPS, corr[:, :],
                    op0=ALU.add, op1=ALU.subtract)
                zrec = wp.tile([128, H], f32, tag="zrec")
                nc.vector.reciprocal(zrec[:, :], zfix[:, :])

                # G2[dst, k, f] = h * w   (one all-bf16 op for the block)
                G2 = wp.tile([128, K, 128], bf16d, tag="G2")
                nc.vector.tensor_tensor(
                    G2[:, :, :].rearrange("p k (h f) -> p k h f", h=H),
                    hcp[:, :, :].rearrange("p k (h f) -> p k h f", h=H),
                    wexb[:, :, :].rearrange("p h k -> p k h").unsqueeze(3)
                        .broadcast_to((128, K, H, HD)),
                    op=ALU.mult)

                # agg = sum_k G2 via halving tree (level 0 bf16, rest f32)
                agg = wp.tile([128, 128], f32, tag="agg")
                src_t, width, li = G2, K, 0
                while width > 1:
                    half = width // 2
                    odd = width - 2 * half
                    if half == 1 and odd == 0:
                        dst_t = agg[:, :].unsqueeze(1)
                    else:
                        dt = bf16d if li == 0 and half > 2 else f32
                        tg = f"tr{li}b" if dt == bf16d else f"tr{li}f"
                        t = wp.tile([128, half + odd, 128], dt, tag=tg)
                        dst_t = t[:, :, :]
                    nc.vector.tensor_tensor(
                        dst_t[:, 0:half, :], src_t[:, 0:half, :],
                        src_t[:, half : 2 * half, :], op=ALU.add)
                    if odd:
                        nc.vector.tensor_copy(dst_t[:, half : half + 1, :],
                                              src_t[:, 2 * half : 2 * half + 1, :])
                    src_t, width, li = dst_t, half + odd, li + 1
                if K == 1:
                    nc.vector.tensor_copy(agg[:, :], G2[:, 0, :])

                na = wp.tile([128, 128], bf16d, tag="na")
                nc.vector.tensor_tensor(
                    na[:, :].rearrange("p (h f) -> p h f", h=H),
                    agg[:, :].rearrange("p (h f) -> p h f", h=H),
                    zrec[:, :].unsqueeze(2).broadcast_to((128, H, HD)),
                    op=ALU.mult)

                # out rows = (na @ wout)
                pt = psT.tile([128, 128], bf16d, tag="pt")
                nc.tensor.transpose(pt[:, :], na[:, :], ident_sb[:, :])
                naT = wp.tile([128, 128], bf16d, tag="naT")
                nc.vector.tensor_copy(naT[:, :], pt[:, :])
                po = psT.tile([128, OUTD], f32, tag="po")
                nc.tensor.matmul(po[:, :], naT[:, :], wout_sb[:, :],
                                 start=True, stop=True)
                ot = op_.tile([128, OUTD], f32, tag="ot")
                nc.vector.tensor_copy(ot[:, :], po[:, :])
                nc.sync.dma_start(out[b * 128 : (b + 1) * 128, :], ot[:, :])

    nc.compile()
    return nc


# ---------------------------------------------------------------- entry point
def kernel(x, edge_index, mask, W, a_src, a_dst, W_out, _cache={}):
    per_core, meta = _host_prep(x, edge_index, mask, W, a_src, a_dst, W_out)
    key = (meta["tot"], tuple(int(k) for k in meta["Kb"]))
    if key not in _cache:
        _cache[key] = _build_nc(meta)
    nc = _cache[key]
    res = run_bass_kernel_spmd(nc, per_core, core_ids=list(range(NCORES)))
    out_new = np.concatenate([res.results[c]["out"] for c in range(NCORES)], axis=0)
    return out_new[meta["pi"]].astype(np.float32)


if __name__ == "__main__":
    rng = np.random.default_rng(0)
    x = rng.standard_normal((N, D)).astype(np.float32)
    ei = rng.integers(0, N, size=(2, E)).astype(np.int32)
    mask = np.ones((N,), bool)
    W = (rng.standard_normal((H, D, HD)) * 0.05).astype(np.float32)
    a_s = (rng.standard_normal((H, HD)) * 0.1).astype(np.float32)
    a_d = (rng.standard_normal((H, HD)) * 0.1).astype(np.float32)
    W_o = (rng.standard_normal((H * HD, OUTD)) * 0.05).astype(np.float32)
    out = kernel(x, ei, mask, W, a_s, a_d, W_o)
    print("ok", out.shape, out.dtype)


# revision 8
# speedup vs baseline: 5.4475x; 1.0610x over previous
"""GAT (graph attention) message-passing kernel for Trainium2, 8 NeuronCores.

v5 strategy: dst-major edge grid, no device-side gather, bf16 bulk vector ops.

Host relabels nodes by descending in-degree and deals them round-robin to the
8 cores, so every core sees the same degree profile. Each core's 12544 dst
nodes form 98 blocks of 128; block b holds nodes of similar degree, padded to
a common per-block edge count K_b (the max degree in the block, identical
across cores). The host pre-gathers x[src] for every edge into a feature-major
DRAM grid xeT[128, TOT] whose column (block, k, seg) is the k-th in-edge of
dst node seg of that block. Pad columns are zero, so padded edges contribute
exactly zero to the aggregation; only the softmax normalizer Z needs a
closed-form correction  Z -= (K - deg) * exp(leaky(s_dst)).

Device work per block: one matmul per k-tile projects 128 edges' source
features into [dst, h(128)|s_src(4)] PSUM rows (dst on partitions). The
Scalar engine copies h to SBUF bf16 (freeing PSUM early) and computes exp;
Vector does scores, one whole-block bf16 weighted-message multiply, and a
contiguous halving-tree k-reduction. No one-hot matmuls, no dma_gather,
no GpSimd.
"""
import sys

sys.path.insert(0, "/opt/trn_rl_repo")

import numpy as np
import ml_dtypes

from concourse import bacc, mybir, tile
from concourse.bass_utils import run_bass_kernel_spmd

f32 = mybir.dt.float32
bf16d = mybir.dt.bfloat16
AF = mybir.ActivationFunctionType
ALU = mybir.AluOpType
BF = ml_dtypes.bfloat16

N = 100000
E = 1600000
D = 128            # in dim
H = 4              # heads
HD = 32            # head dim
OUTD = 128
NEG = 0.2
EPS = 1e-8

NCORES = 8
BLK = 98
NPB = BLK * 128          # 12544 dst nodes per core
NPAD = NCORES * NPB      # 100352
GRP = 3                  # h-matmuls per PSUM bank tile (3*132 words <= 512)


# ---------------------------------------------------------------- host prep
def _host_prep(x, edge_index, mask, W, a_src, a_dst, W_out):
    x = np.asarray(x, np.float32)
    src = np.asarray(edge_index[0], np.int64)
    dst = np.asarray(edge_index[1], np.int64)
    keep = np.asarray(mask, bool)[src]
    if not keep.all():
        src, dst = src[keep], dst[keep]

    deg = np.bincount(dst, minlength=N).astype(np.int64)
    degp = np.full(NPAD, -1, np.int64)
    degp[:N] = deg
    order = np.argsort(-degp, kind="stable")
    rank = np.empty(NPAD, np.int64)
    rank[order] = np.arange(NPAD)
    core_of = rank % NCORES
    pos = rank // NCORES

    deg_sorted = degp[order]
    Kb = np.maximum(deg_sorted[np.arange(BLK) * 128 * NCORES], 1).astype(np.int64)
    assert Kb.max() <= 64, Kb.max()
    colstart = np.concatenate([[0], np.cumsum(Kb * 128)]).astype(np.int64)
    TOT = int(colstart[-1])

    # k-rank of each edge within its dst node
    o2 = np.argsort(dst, kind="stable")
    ds, ss = dst[o2], src[o2]
    first = np.r_[True, ds[1:] != ds[:-1]]
    gstart = np.flatnonzero(first)
    glen = np.diff(np.r_[gstart, [len(ds)]])
    krank = np.arange(len(ds)) - np.repeat(gstart, glen)
    b_e = pos[ds] // 128
    s_e = pos[ds] % 128
    c_e = core_of[ds]
    col = colstart[b_e] + krank * 128 + s_e

    xbf = x.astype(BF)
    node_at = np.full((NCORES, NPB), N, np.int64)   # default: pad slot
    node_at[core_of, pos] = np.arange(NPAD)
    xpad = np.zeros((NPAD, D), np.float32)
    xpad[:N] = x
    deg_pad = np.zeros(NPAD, np.float32)
    deg_pad[:N] = deg

    W_cat = np.asarray(W, np.float32).transpose(1, 0, 2).reshape(D, H * HD)
    A_src = np.zeros((H * HD, H), np.float32)
    A_dst = np.zeros((H * HD, H), np.float32)
    for h in range(H):
        A_src[h * HD:(h + 1) * HD, h] = np.asarray(a_src, np.float32)[h]
        A_dst[h * HD:(h + 1) * HD, h] = np.asarray(a_dst, np.float32)[h]
    wcat_ext = np.ascontiguousarray(
        np.concatenate([W_cat, W_cat @ A_src], axis=1)).astype(BF)  # [128, 132]
    mdst_h = np.ascontiguousarray(W_cat @ A_dst).astype(BF)          # [128, 4]
    wout_h = np.asarray(W_out, np.float32).astype(BF)
    ident_h = np.eye(128, dtype=np.float32).astype(BF)

    per_core = []
    for c in range(NCORES):
        sel = c_e == c
        xeT = np.zeros((D, TOT), BF)
        xeT[:, col[sel]] = xbf[ss[sel]].T
        nodes = node_at[c]
        xtd = np.ascontiguousarray(xpad[nodes].T).astype(BF)           # [128, NPB]
        # K_b - deg per (seg, block): pad count for the Z correction
        kmd = (Kb[None, :].astype(np.float32)
               - deg_pad[nodes].reshape(BLK, 128).T)                   # [128, BLK]
        per_core.append(dict(
            xeT=xeT, xtd=xtd, kmd=np.ascontiguousarray(kmd).astype(np.float32),
            wcat_ext=wcat_ext, mdst=mdst_h, wout=wout_h, ident=ident_h,
        ))

    pi = (core_of * NPB + pos)[:N]
    meta = dict(Kb=Kb, colstart=colstart, tot=TOT, pi=pi)
    return per_core, meta


# ---------------------------------------------------------------- device build
def _build_nc(meta):
    Kb = [int(k) for k in meta["Kb"]]
    colstart = [int(v) for v in meta["colstart"]]
    TOT = int(meta["tot"])

    nc = bacc.Bacc(None, target_bir_lowering=False)
    xeT = nc.dram_tensor("xeT", [D, TOT], bf16d, kind="ExternalInput")
    xtd = nc.dram_tensor("xtd", [D, NPB], bf16d, kind="ExternalInput")
    kmd = nc.dram_tensor("kmd", [128, BLK], f32, kind="ExternalInput")
    wcat_ext = nc.dram_tensor("wcat_ext", [D, 132], bf16d, kind="ExternalInput")
    mdst = nc.dram_tensor("mdst", [D, H], bf16d, kind="ExternalInput")
    wout = nc.dram_tensor("wout", [H * HD, OUTD], bf16d, kind="ExternalInput")
    ident = nc.dram_tensor("ident", [128, 128], bf16d, kind="ExternalInput")
    out = nc.dram_tensor("out", [NPB, OUTD], f32, kind="ExternalOutput")

    with tile.TileContext(nc) as tc:
        with (
            tc.tile_pool(name="const", bufs=1) as cpool,
            tc.tile_pool(name="xe", bufs=3) as xp,
            tc.tile_pool(name="work", bufs=2) as wp,
            tc.tile_pool(name="outp", bufs=3) as op_,
            tc.tile_pool(name="psH", bufs=5, space="PSUM") as psH,
            tc.tile_pool(name="psK", bufs=1, space="PSUM") as psK,
            tc.tile_pool(name="psT", bufs=1, space="PSUM") as psT,
        ):
            wcat_sb = cpool.tile([D, 132], bf16d)
            nc.sync.dma_start(wcat_sb[:, :], wcat_ext[:, :])
            mdst_sb = cpool.tile([D, H], bf16d)
            nc.sync.dma_start(mdst_sb[:, :], mdst[:, :])
            wout_sb = cpool.tile([H * HD, OUTD], bf16d)
            nc.sync.dma_start(wout_sb[:, :], wout[:, :])
            ident_sb = cpool.tile([128, 128], bf16d)
            nc.sync.dma_start(ident_sb[:, :], ident[:, :])
            xtd_sb = cpool.tile([D, NPB], bf16d)
            nc.sync.dma_start(xtd_sb[:, :], xtd[:, :])
            kmd_sb = cpool.tile([128, BLK], f32)
            nc.sync.dma_start(kmd_sb[:, :], kmd[:, :])

            for b in range(BLK):
                K = Kb[b]
                c0b = colstart[b]

                xe = xp.tile([128, K * 128], bf16d, tag="xe")
                nc.sync.dma_start(xe[:, :], xeT[:, c0b : c0b + K * 128])

                # s_dst for the block's 128 dst nodes
                ps_sd = psK.tile([128, H], f32, tag="sd")
                nc.tensor.matmul(ps_sd[:, :], xtd_sb[:, b * 128 : (b + 1) * 128],
                                 mdst_sb[:, :], start=True, stop=True)
                sdst = wp.tile([128, H], f32, tag="sdst")
                nc.scalar.activation(sdst[:, :], ps_sd[:, :], AF.Copy)

                esum = wp.tile([128, H, K], f32, tag="esum")
                hcp = wp.tile([128, K, 128], bf16d, tag="hcp")

                for g0 in range(0, K, GRP):
                    m = min(GRP, K - g0)
                    ps = psH.tile([128, GRP * 132], f32, tag="ps")
                    for j in range(m):
                        k = g0 + j
                        nc.tensor.matmul(
                            ps[:, j * 132 : (j + 1) * 132],
                            xe[:, k * 128 : (k + 1) * 128],
                            wcat_sb[:, :], start=True, stop=True)
                    psv = ps[:, :].rearrange("p (j f) -> p j f", f=132)
                    # e = s_src + s_dst, straight off PSUM
                    nc.vector.tensor_tensor(
                        esum[:, :, g0 : g0 + m],
                        psv[:, 0:m, 128:132].rearrange("p j h -> p h j"),
                        sdst[:, :].unsqueeze(2).broadcast_to((128, H, m)),
                        op=ALU.add)
                    # h rows to SBUF bf16 (frees PSUM)
                    nc.scalar.activation(
                        hcp[:, g0 : g0 + m, :], psv[:, 0:m, 0:128], AF.Copy)

                # w = exp(leaky(e)) ; wb = bf16(w) ; Z = sum_k w
                lrt = wp.tile([128, H, K], f32, tag="lrt")
                nc.vector.scalar_tensor_tensor(
                    lrt[:, :, :], esum[:, :, :], NEG, esum[:, :, :],
                    op0=ALU.mult, op1=ALU.max)
                wex = wp.tile([128, H, K], f32, tag="wex")
                nc.scalar.activation(wex[:, :, :], lrt[:, :, :], AF.Exp)
                wexb = wp.tile([128, H, K], bf16d, tag="wexb")
                nc.scalar.activation(wexb[:, :, :], wex[:, :, :], AF.Copy)
                zall = wp.tile([128, H], f32, tag="zall")
                nc.vector.tensor_reduce(zall[:, :], wex[:, :, :],
                                        mybir.AxisListType.X, ALU.add)

                # Z -= (K-deg)*exp(leaky(s_dst)); zrec = 1/(Z+eps)
                lrp = wp.tile([128, H], f32, tag="lrp")
                nc.vector.scalar_tensor_tensor(
                    lrp[:, :], sdst[:, :], NEG, sdst[:, :],
                    op0=ALU.mult, op1=ALU.max)
                wpad = wp.tile([128, H], f32, tag="wpad")
                nc.scalar.activation(wpad[:, :], lrp[:, :], AF.Exp)
                corr = wp.tile([128, H], f32, tag="corr")
                nc.vector.tensor_scalar(corr[:, :], wpad[:, :],
                                        kmd_sb[:, b : b + 1], None, op0=ALU.mult)
                zfix = wp.tile([128, H], f32, tag="zfix")
                nc.vector.scalar_tensor_tensor(
                    zfix[:, :], zall[:, :], EPS, corr[:, :],
                    op0=ALU.add, op1=ALU.subtract)
                zrec = wp.tile([128, H], f32, tag="zrec")
                nc.vector.reciprocal(zrec[:, :], zfix[:, :])

                # G2[dst, k, f] = h * w   (one all-bf16 op for the block)
                G2 = wp.tile([128, K, 128], bf16d, tag="G2")
                nc.vector.tensor_tensor(
                    G2[:, :, :].rearrange("p k (h f) -> p k h f", h=H),
                    hcp[:, :, :].rearrange("p k (h f) -> p k h f", h=H),
                    wexb[:, :, :].rearrange("p h k -> p k h").unsqueeze(3)
                        .broadcast_to((128, K, H, HD)),
                    op=ALU.mult)

                # agg = sum_k G2 via halving tree (bf16 2x levels, final to f32)
                agg = wp.tile([128, 128], f32, tag="agg")
                src_t, width, li = G2, K, 0
                while width > 1:
                    half = width // 2
                    odd = width - 2 * half
                    if half == 1 and odd == 0:
                        dst_t = agg[:, :].unsqueeze(1)
                    else:
                        t = wp.tile([128, half + odd, 128], bf16d, tag=f"tr{li}")
                        dst_t = t[:, :, :]
                    nc.vector.tensor_tensor(
                        dst_t[:, 0:half, :], src_t[:, 0:half, :],
                        src_t[:, half : 2 * half, :], op=ALU.add)
                    if odd:
                        nc.scalar.activation(dst_t[:, half : half + 1, :],
                                             src_t[:, 2 * half : 2 * half + 1, :],
                                             AF.Copy)
                    src_t, width, li = dst_t, half + odd, li + 1
                if K == 1:
                    nc.vector.tensor_copy(agg[:, :], G2[:, 0, :])

                na = wp.tile([128, 128], bf16d, tag="na")
                nc.vector.tensor_tensor(
                    na[:, :].rearrange("p (h f) -> p h f", h=H),
                    agg[:, :].rearrange("p (h f) -> p h f", h=H),
                    zrec[:, :].unsqueeze(2).broadcast_to((128, H, HD)),
                    op=ALU.mult)

                # out rows = (na @ wout)
                pt = psT.tile([128, 128], bf16d, tag="pt")
                nc.tensor.transpose(pt[:, :], na[:, :], ident_sb[:, :])
                naT = wp.tile([128, 128], bf16d, tag="naT")
                nc.scalar.activation(naT[:, :], pt[:, :], AF.Copy)
                po = psT.tile([128, OUTD], f32, tag="po")
                nc.tensor.matmul(po[:, :], naT[:, :], wout_sb[:, :],
                                 start=True, stop=True)
                ot = op_.tile([128, OUTD], f32, tag="ot")
                nc.scalar.activation(ot[:, :], po[:, :], AF.Copy)
                nc.sync.dma_start(out[b * 128 : (b + 1) * 128, :], ot[:, :])

    nc.compile()
    return nc


# ---------------------------------------------------------------- entry point
def kernel(x, edge_index, mask, W, a_src, a_dst, W_out, _cache={}):
    per_core, meta = _host_prep(x, edge_index, mask, W, a_src, a_dst, W_out)
    key = (meta["tot"], tuple(int(k) for k in meta["Kb"]))
    if key not in _cache:
        _cache[key] = _build_nc(meta)
    nc = _cache[key]
    res = run_bass_kernel_spmd(nc, per_core, core_ids=list(range(NCORES)))
    out_new = np.concatenate([res.results[c]["out"] for c in range(NCORES)], axis=0)
    return out_new[meta["pi"]].astype(np.float32)


if __name__ == "__main__":
    rng = np.random.default_rng(0)
    x = rng.standard_normal((N, D)).astype(np.float32)
    ei = rng.integers(0, N, size=(2, E)).astype(np.int32)
    mask = np.ones((N,), bool)
    W = (rng.standard_normal((H, D, HD)) * 0.05).astype(np.float32)
    a_s = (rng.standard_normal((H, HD)) * 0.1).astype(np.float32)
    a_d = (rng.standard_normal((H, HD)) * 0.1).astype(np.float32)
    W_o = (rng.standard_normal((H * HD, OUTD)) * 0.05).astype(np.float32)
    out = kernel(x, ei, mask, W, a_s, a_d, W_o)
    print("ok", out.shape, out.dtype)
